# revision 8
# baseline (speedup 1.0000x reference)
"""BiLSTM-CRF Trainium2 kernel (8 NeuronCores, SPMD data-parallel over batch).

Reference model: emb lookup -> BiLSTM (H2=512 each dir) -> linear [1024->12]
-> per-sentence Viterbi decode (12 tags). B=64, S=512, E=512, V=50000.

Device strategy per core (8 sentences/core, both directions on-core):
  P1  x-projection: g_x = x @ W_ih.T + b (both dirs) -> DRAM, f32r matmuls
  P2  recurrence: per step h-stationary f32r matmuls (lhsT = h.T chunks
      [128,8], rhs = W_hh.T streamed), gates on ACT/DVE in [8,2048] layout,
      h transposed back via PE transposes; fwd+bwd chains interleaved by the
      Tile scheduler; h.T history spilled to DRAM
  P3  output projection: feats = [hf|hb] @ W_out.T + b_out (b_out folded into
      trans on host) -> DRAM
  P4  Viterbi DP (max-plus scan) in [batch-on-partition, (i,j) free] layout
  P5  backpointers recomputed in parallel, resharded to 128 partitions
Host: embedding gather, weight layout prep, final argmax/backtrace.
"""
import os
import numpy as np
from contextlib import ExitStack

import concourse.bass as bass
import concourse.tile as tile
from concourse import bass_utils, bacc, mybir
from concourse.bass import ds

V, E, H, Tg = 50000, 512, 1024, 12
H2 = H // 2
START, STOP = 10, 11
NEG = -10000.0
B, S = 64, 512
NCORES = 8
BL = B // NCORES          # 8 sentences per core
G4 = 4 * H2               # 2048 gate dim
BT = BL * S               # 4096 (t-major: bt = t*BL + b)
F32 = mybir.dt.float32
F32R = mybir.dt.float32r
AF = mybir.ActivationFunctionType
OP = mybir.AluOpType

# gate order in our layout: [i, f, o, g] (sigmoid on first 3*H2, tanh on last)
SL_I = slice(0, H2)
SL_F = slice(H2, 2 * H2)
SL_O = slice(2 * H2, 3 * H2)
SL_G = slice(3 * H2, 4 * H2)


def build_nc(s_dev=S, u_unroll=2, staggered=False):
    """Build the device program for sequence length s_dev (must divide by 2*u)."""
    bt = BL * s_dev
    nc = bacc.Bacc("TRN2", target_bir_lowering=False, debug=False)

    # ---- I/O -------------------------------------------------------------
    xT_d = nc.dram_tensor("xT", [E, bt], F32R, kind="ExternalInput")
    WihT_d = nc.dram_tensor("WihT", [2 * E, G4], F32R, kind="ExternalInput")
    WhhT_d = nc.dram_tensor("WhhT", [2 * H2, G4], F32R, kind="ExternalInput")
    bias_d = nc.dram_tensor("biasg", [2, G4], F32R, kind="ExternalInput")
    ones_d = nc.dram_tensor("ones1", [1, 128], F32R, kind="ExternalInput")
    id8_d = nc.dram_tensor("id8", [BL, BL], F32R, kind="ExternalInput")
    WoT_d = nc.dram_tensor("WoT", [2 * H2, Tg], F32R, kind="ExternalInput")
    trans_d = nc.dram_tensor("transf", [128, Tg * Tg], F32, kind="ExternalInput")
    revio_d = nc.dram_tensor("revio", [128, Tg * Tg], F32, kind="ExternalInput")
    fv0_d = nc.dram_tensor("fv0", [BL, Tg], F32, kind="ExternalInput")

    fvh_d = nc.dram_tensor("fvh", [BL, s_dev * Tg], F32, kind="ExternalOutput")
    fvfin_d = nc.dram_tensor("fvfin", [BL, Tg], F32, kind="ExternalOutput")
    # bp partitions: p = b*16 + tc  (tchunks of s_dev/16)
    tcs = s_dev // 16                      # t per chunk
    bp_d = nc.dram_tensor("bp", [128, tcs * Tg], F32, kind="ExternalOutput")

    # ---- scratch ---------------------------------------------------------
    gx_d = nc.dram_tensor("gx_scr", [2, bt, G4], F32R)
    hT_d = nc.dram_tensor("hT_scr", [2, 4, 128, bt], F32R)
    feats_d = nc.dram_tensor("feats_scr", [bt, Tg], F32)

    with tile.TileContext(nc) as tc:
        with ExitStack() as octx:
            # persistent across phases
            perm = octx.enter_context(tc.tile_pool(name="perm", bufs=1))
            WhhT = perm.tile([128, 4 * G4 * 2], F32R)   # [k*2048 + g] per dir halves
            for d in range(2):
                for k in range(4):
                    nc.sync.dma_start(
                        WhhT[:, d * 4 * G4 + k * G4:(d * 4 * G4) + (k + 1) * G4],
                        WhhT_d.ap()[d * H2 + k * 128: d * H2 + (k + 1) * 128, :])
            ones1 = perm.tile([1, 128], F32R)
            nc.sync.dma_start(ones1[:], ones_d.ap())
            id8 = perm.tile([BL, BL], F32R)
            nc.sync.dma_start(id8[:], id8_d.ap())

            # ================= P1: x-projection ===========================
            with ExitStack() as ctx:
                xpool = ctx.enter_context(tc.tile_pool(name="p1x", bufs=1))
                wpool = ctx.enter_context(tc.tile_pool(name="p1w", bufs=1))
                spool = ctx.enter_context(tc.tile_pool(name="p1s", bufs=3))
                ppool = ctx.enter_context(tc.tile_pool(name="p1p", bufs=2, space="PSUM"))

                WihT = wpool.tile([128, 4 * G4 * 2], F32R)
                for d in range(2):
                    for k in range(4):
                        nc.sync.dma_start(
                            WihT[:, d * 4 * G4 + k * G4: d * 4 * G4 + (k + 1) * G4],
                            WihT_d.ap()[d * E + k * 128: d * E + (k + 1) * 128, :])
                biasg = wpool.tile([1, 2 * G4], F32R)
                nc.sync.dma_start(biasg[:], bias_d.ap().rearrange("a b -> (a b)").unsqueeze(0))

                for btc in range(bt // 128):
                    xts = []
                    for k in range(4):
                        xt = xpool.tile([128, 128], F32R, tag="xts", bufs=8)
                        nc.sync.dma_start(xt[:], xT_d.ap()[k * 128:(k + 1) * 128,
                                                           btc * 128:(btc + 1) * 128])
                        xts.append(xt)
                    for d in range(2):
                        for n in range(4):
                            ps = ppool.tile([128, 512], F32, tag="xp")
                            for k in range(4):
                                nc.tensor.matmul(
                                    ps[:],
                                    xts[k][:],
                                    WihT[:, d * 4 * G4 + k * G4 + n * 512: d * 4 * G4 + k * G4 + (n + 1) * 512],
                                    start=(k == 0), stop=False)
                            nc.tensor.matmul(
                                ps[:], ones1[:, 0:128],
                                biasg[:, d * G4 + n * 512: d * G4 + (n + 1) * 512],
                                start=False, stop=True)
                            gxs = spool.tile([128, 512], F32R, tag="gxs")
                            nc.scalar.copy(gxs[:], ps[:])
                            nc.sync.dma_start(
                                gx_d.ap()[d, btc * 128:(btc + 1) * 128, n * 512:(n + 1) * 512],
                                gxs[:])

            # ================= P2: recurrence =============================
            with ExitStack() as ctx:
                st = ctx.enter_context(tc.tile_pool(name="p2st", bufs=1))
                gxp = ctx.enter_context(tc.tile_pool(name="p2gx", bufs=2))
                gp = ctx.enter_context(tc.tile_pool(name="p2g", bufs=2))
                tp = ctx.enter_context(tc.tile_pool(name="p2t", bufs=2))
                hp = ctx.enter_context(tc.tile_pool(name="p2h", bufs=2))
                pp = ctx.enter_context(tc.tile_pool(name="p2p", bufs=1, space="PSUM"))

                hT = [st.tile([128, 4 * BL], F32R, name=f"hT{d}") for d in range(2)]
                c_t = [st.tile([BL, H2], F32, name=f"ct{d}") for d in range(2)]
                for d in range(2):
                    nc.vector.memset(hT[d][:].bitcast(F32), 0.0)
                    nc.vector.memset(c_t[d][:], 0.0)

                def step(d, t_expr):
                    """one LSTM step for direction d at time t (ScalarValue or int)."""
                    gxt = gxp.tile([BL, G4], F32R, tag=f"gx{d}")
                    nc.sync.dma_start(gxt[:], gx_d.ap()[d, ds(t_expr * BL, BL), :])
                    ps = pp.tile([BL, G4], F32, tag=f"ps{d}")
                    for n in range(4):
                        for k in range(4):
                            nc.tensor.matmul(
                                ps[:, n * 512:(n + 1) * 512],
                                hT[d][:, k * BL:(k + 1) * BL],
                                WhhT[:, d * 4 * G4 + k * G4 + n * 512: d * 4 * G4 + k * G4 + (n + 1) * 512],
                                start=(k == 0), stop=(k == 3))
                    g = gp.tile([BL, G4], F32, tag=f"g{d}")
                    nc.vector.tensor_add(g[:], ps[:], gxt[:])
                    nc.scalar.activation(g[:, 0:3 * H2], g[:, 0:3 * H2], AF.Sigmoid)
                    nc.scalar.activation(g[:, SL_G], g[:, SL_G], AF.Tanh)
                    tmp = tp.tile([BL, H2], F32, tag=f"tmp{d}")
                    nc.vector.tensor_mul(tmp[:], g[:, SL_I], g[:, SL_G])
                    nc.vector.tensor_mul(c_t[d][:], g[:, SL_F], c_t[d][:])
                    nc.vector.tensor_add(c_t[d][:], c_t[d][:], tmp[:])
                    tanhc = tp.tile([BL, H2], F32, tag=f"th{d}")
                    nc.scalar.activation(tanhc[:], c_t[d][:], AF.Tanh)
                    h = hp.tile([BL, H2], F32R, tag=f"h{d}")
                    nc.vector.tensor_mul(h[:], g[:, SL_O], tanhc[:])
                    psT = pp.tile([128, 4 * BL], F32R, tag=f"ps{d}")
                    for k in range(4):
                        nc.tensor.transpose(psT[:, k * BL:(k + 1) * BL],
                                            h[:, k * 128:(k + 1) * 128], id8[:])
                    nc.scalar.copy(hT[d][:], psT[:])
                    for k in range(4):
                        nc.sync.dma_start(
                            hT_d.ap()[d, k, :, ds(t_expr * BL, BL)],
                            hT[d][:, k * BL:(k + 1) * BL])

                niter = s_dev // u_unroll
                with tc.For_i(0, niter, 1, staggered_reset=staggered) as i:
                    for u in range(u_unroll):
                        step(0, i * u_unroll + u)
                        step(1, (s_dev - 1) - (i * u_unroll + u))

            # ================= P3: output projection ======================
            with ExitStack() as ctx:
                wpool = ctx.enter_context(tc.tile_pool(name="p3w", bufs=1))
                lpool = ctx.enter_context(tc.tile_pool(name="p3l", bufs=3))
                spool = ctx.enter_context(tc.tile_pool(name="p3s", bufs=3))
                ppool = ctx.enter_context(tc.tile_pool(name="p3p", bufs=2, space="PSUM"))

                WoT = wpool.tile([128, 8 * Tg], F32R)      # (d*4+k) at cols *Tg
                for d in range(2):
                    for k in range(4):
                        nc.sync.dma_start(
                            WoT[:, (d * 4 + k) * Tg:(d * 4 + k + 1) * Tg],
                            WoT_d.ap()[d * H2 + k * 128: d * H2 + (k + 1) * 128, :])

                for btc in range(bt // 128):
                    ps = ppool.tile([128, Tg], F32, tag="fp")
                    for d in range(2):
                        for k in range(4):
                            hTl = lpool.tile([128, 128], F32R, tag="hTl")
                            nc.sync.dma_start(
                                hTl[:], hT_d.ap()[d, k, :, btc * 128:(btc + 1) * 128])
                            nc.tensor.matmul(ps[:], hTl[:],
                                             WoT[:, (d * 4 + k) * Tg:(d * 4 + k + 1) * Tg],
                                             start=(d == 0 and k == 0), stop=(d == 1 and k == 3))
                    fe = spool.tile([128, Tg], F32, tag="fe")
                    nc.scalar.copy(fe[:], ps[:])
                    nc.sync.dma_start(feats_d.ap()[btc * 128:(btc + 1) * 128, :], fe[:])

            # ================= P4: Viterbi DP =============================
            with ExitStack() as ctx:
                vp = ctx.enter_context(tc.tile_pool(name="p4", bufs=1))
                vt = ctx.enter_context(tc.tile_pool(name="p4t", bufs=1))

                feats_sb = vp.tile([BL, s_dev * Tg], F32)
                nc.sync.dma_start(
                    feats_sb[:].rearrange("b (t i) -> b t i", i=Tg),
                    feats_d.ap().rearrange("(t b) i -> b t i", b=BL))
                trans_sb = vp.tile([128, Tg * Tg], F32)
                nc.sync.dma_start(trans_sb[:], trans_d.ap())
                revio_sb = vp.tile([128, Tg * Tg], F32)
                nc.sync.dma_start(revio_sb[:], revio_d.ap())
                fv = vp.tile([BL, Tg], F32)
                nc.sync.dma_start(fv[:], fv0_d.ap())
                fvh_sb = vp.tile([BL, s_dev * Tg], F32)

                uv = 8
                with tc.For_i(0, s_dev // uv, 1) as i:
                    for u in range(uv):
                        t = i * uv + u
                        nc.scalar.copy(fvh_sb[:, ds(t * Tg, Tg)], fv[:])
                        nv = vt.tile([BL, Tg * Tg], F32, tag="nv")
                        nc.vector.tensor_tensor(
                            nv[:].rearrange("b (i j) -> b i j", i=Tg),
                            fv[:].unsqueeze(1).broadcast_to([BL, Tg, Tg]),
                            trans_sb[0:BL, :].rearrange("b (i j) -> b i j", i=Tg),
                            OP.add)
                        nc.vector.tensor_reduce(
                            fv[:], nv[:].rearrange("b (i j) -> b i j", i=Tg),
                            mybir.AxisListType.X, OP.max)
                        nc.vector.tensor_add(fv[:], fv[:], feats_sb[:, ds(t * Tg, Tg)])
                nc.sync.dma_start(fvh_d.ap(), fvh_sb[:])
                nc.sync.dma_start(fvfin_d.ap(), fv[:])

                # ============= P5: backpointers (resharded) ===============
                fvp = vp.tile([128, tcs * Tg], F32)     # p = b*16+tc
                for b in range(BL):
                    nc.sync.dma_start(
                        fvp[b * 16:(b + 1) * 16, :],
                        fvh_d.ap().rearrange("b (tc r) -> b tc r", tc=16)[b, :, :])
                nva = vt.tile([128, tcs * Tg * Tg], F32, tag="nva")
                nc.vector.tensor_tensor(
                    nva[:].rearrange("p (t i j) -> p t i j", t=tcs, i=Tg),
                    fvp[:].rearrange("p (t j) -> p t j", t=tcs).unsqueeze(2).broadcast_to([128, tcs, Tg, Tg]),
                    trans_sb[:].rearrange("p (i j) -> p i j", i=Tg).unsqueeze(1).broadcast_to([128, tcs, Tg, Tg]),
                    OP.add)
                mx = vt.tile([128, tcs * Tg], F32, tag="mx")
                nc.vector.tensor_reduce(
                    mx[:], nva[:].rearrange("p (t i j) -> p t i j", t=tcs, i=Tg),
                    mybir.AxisListType.X, OP.max)
                eq = vt.tile([128, tcs * Tg * Tg], F32, tag="eq")
                nc.vector.tensor_tensor(
                    eq[:].rearrange("p (t i j) -> p t i j", t=tcs, i=Tg),
                    nva[:].rearrange("p (t i j) -> p t i j", t=tcs, i=Tg),
                    mx[:].rearrange("p (t i) -> p t i", t=tcs).unsqueeze(3).broadcast_to([128, tcs, Tg, Tg]),
                    OP.is_equal)
                nc.vector.tensor_tensor(
                    eq[:].rearrange("p (t i j) -> p t i j", t=tcs, i=Tg),
                    eq[:].rearrange("p (t i j) -> p t i j", t=tcs, i=Tg),
                    revio_sb[:].rearrange("p (i j) -> p i j", i=Tg).unsqueeze(1).broadcast_to([128, tcs, Tg, Tg]),
                    OP.mult)
                bpv = vt.tile([128, tcs * Tg], F32, tag="bpv")
                nc.vector.tensor_reduce(
                    bpv[:], eq[:].rearrange("p (t i j) -> p t i j", t=tcs, i=Tg),
                    mybir.AxisListType.X, OP.max)
                nc.sync.dma_start(bp_d.ap(), bpv[:])

    nc.compile()
    return nc


# ---------------------------------------------------------------------------
_NC_CACHE = {}


def _get_nc(s_dev=S):
    if s_dev not in _NC_CACHE:
        _NC_CACHE[s_dev] = build_nc(s_dev)
    return _NC_CACHE[s_dev]


def host_prep(inputs, s_dev=S):
    """Build per-core in_maps from full inputs."""
    sent = np.asarray(inputs["sentences"])
    emb = np.asarray(inputs["emb"], dtype=np.float32)
    f32 = lambda k: np.asarray(inputs[k], dtype=np.float32)
    W_ih = [f32("W_ih_f"), f32("W_ih_b")]
    W_hh = [f32("W_hh_f"), f32("W_hh_b")]
    b_g = [f32("b_f"), f32("b_b")]
    W_out = f32("W_out")
    b_out = f32("b_out")
    trans = f32("trans")

    # gate perm [i, f, g, o] -> [i, f, o, g]
    def gperm(M):
        return np.concatenate([M[:H2], M[H2:2 * H2], M[3 * H2:], M[2 * H2:3 * H2]], axis=0)

    WihT = np.concatenate([gperm(W_ih[d]).T for d in range(2)], axis=0)       # [2E, G4]
    WhhT = np.concatenate([gperm(W_hh[d]).T for d in range(2)], axis=0)       # [2H2, G4]
    biasg = np.stack([gperm(b_g[d].reshape(G4, 1))[:, 0] for d in range(2)])  # [2, G4]
    WoT = np.concatenate([W_out[:, :H2].T, W_out[:, H2:].T], axis=0)          # [2H2, Tg]
    transf = np.tile((trans + b_out[:, None]).reshape(1, -1), (128, 1)).astype(np.float32)
    revio = np.tile(np.tile(Tg - 1.0 - np.arange(Tg, dtype=np.float32), Tg).reshape(1, -1), (128, 1))
    fv0 = np.full((BL, Tg), NEG, np.float32)
    fv0[:, START] = 0.0
    ones1 = np.ones((1, 128), np.float32)
    id8 = np.eye(BL, dtype=np.float32)

    x = emb[sent]                                   # [B, S, E]
    in_maps = []
    for c in range(NCORES):
        xc = x[c * BL:(c + 1) * BL, :s_dev]         # [BL, s, E]
        xT = np.ascontiguousarray(xc.transpose(2, 1, 0).reshape(E, s_dev * BL))
        in_maps.append(dict(
            xT=xT.astype(np.float32), WihT=WihT, WhhT=WhhT, biasg=biasg,
            ones1=ones1, id8=id8, WoT=WoT, transf=transf, revio=revio, fv0=fv0))
    return in_maps, trans


def host_post(results, trans, s_dev=S):
    """Assemble (scores, paths) from per-core outputs."""
    tcs = s_dev // 16
    scores = np.empty((B,), np.float32)
    paths = np.empty((B, s_dev), np.int32)
    for c, r in enumerate(results):
        fvfin = r["fvfin"]                          # [BL, Tg]
        bpv = r["bp"]                               # [128, tcs*Tg]
        bp = np.empty((BL, s_dev, Tg), np.int64)
        for b in range(BL):
            for tchunk in range(16):
                blk = bpv[b * 16 + tchunk].reshape(tcs, Tg)
                bp[b, tchunk * tcs:(tchunk + 1) * tcs] = (Tg - 1.0 - blk).astype(np.int64)
        term = fvfin + trans[STOP][None, :]         # [BL, Tg]
        for b in range(BL):
            gb = c * BL + b
            best = int(np.argmax(term[b]))
            scores[gb] = term[b, best]
            tag = best
            for t in range(s_dev - 1, -1, -1):
                paths[gb, t] = tag
                tag = int(bp[b, t, tag])
    return scores, paths


def kernel(**inputs):
    nc = _get_nc(S)
    in_maps, trans = host_prep(inputs, S)
    res = bass_utils.run_bass_kernel_spmd(nc, in_maps, core_ids=list(range(NCORES)))
    return host_post(res.results, trans, S)


# revision 10
# speedup vs baseline: 1.2927x; 1.2927x over previous
"""BiLSTM-CRF Trainium2 kernel (8 NeuronCores, SPMD data-parallel over batch).

Reference model: emb lookup -> BiLSTM (H2=512 each dir) -> linear [1024->12]
-> per-sentence Viterbi decode (12 tags). B=64, S=512, E=512, V=50000.

Device strategy per core (8 sentences/core, both directions on-core):
  P1  x-projection: g_x = x @ W_ih.T + b (both dirs) -> DRAM, f32r matmuls
  P2  recurrence: per step h-stationary f32r matmuls (lhsT = h.T chunks
      [128,8], rhs = W_hh.T streamed), gates on ACT/DVE in [8,2048] layout,
      h transposed back via PE transposes; fwd+bwd chains interleaved by the
      Tile scheduler; h.T history spilled to DRAM
  P3  output projection: feats = [hf|hb] @ W_out.T + b_out (b_out folded into
      trans on host) -> DRAM
  P4  Viterbi DP (max-plus scan) in [batch-on-partition, (i,j) free] layout
  P5  backpointers recomputed in parallel, resharded to 128 partitions
Host: embedding gather, weight layout prep, final argmax/backtrace.
"""
import os
import numpy as np
from contextlib import ExitStack

import concourse.bass as bass
import concourse.tile as tile
from concourse import bass_utils, bacc, mybir
from concourse.bass import ds

V, E, H, Tg = 50000, 512, 1024, 12
H2 = H // 2
START, STOP = 10, 11
NEG = -10000.0
B, S = 64, 512
NCORES = 8
BL = B // NCORES          # 8 sentences per core
G4 = 4 * H2               # 2048 gate dim
BT = BL * S               # 4096 (t-major: bt = t*BL + b)
F32 = mybir.dt.float32
F32R = mybir.dt.float32r
AF = mybir.ActivationFunctionType
OP = mybir.AluOpType

# gate order in our layout: [i, f, o, g] (sigmoid on first 3*H2, tanh on last)
SL_I = slice(0, H2)
SL_F = slice(H2, 2 * H2)
SL_O = slice(2 * H2, 3 * H2)
SL_G = slice(3 * H2, 4 * H2)


def build_nc(s_dev=S, u_unroll=4, staggered=True):
    """Build the device program for sequence length s_dev (must divide by 2*u)."""
    bt = BL * s_dev
    nc = bacc.Bacc("TRN2", target_bir_lowering=False, debug=False)

    # ---- I/O -------------------------------------------------------------
    xT_d = nc.dram_tensor("xT", [E, bt], F32R, kind="ExternalInput")
    WihT_d = nc.dram_tensor("WihT", [2 * E, G4], F32R, kind="ExternalInput")
    WhhT_d = nc.dram_tensor("WhhT", [2 * H2, G4], F32R, kind="ExternalInput")
    bias_d = nc.dram_tensor("biasg", [2, G4], F32R, kind="ExternalInput")
    ones_d = nc.dram_tensor("ones1", [1, 128], F32R, kind="ExternalInput")
    id8_d = nc.dram_tensor("id8", [BL, BL], F32R, kind="ExternalInput")
    WoT_d = nc.dram_tensor("WoT", [2 * H2, Tg], F32R, kind="ExternalInput")
    trans_d = nc.dram_tensor("transf", [128, Tg * Tg], F32, kind="ExternalInput")
    revio_d = nc.dram_tensor("revio", [128, Tg * Tg], F32, kind="ExternalInput")
    fv0_d = nc.dram_tensor("fv0", [BL, Tg], F32, kind="ExternalInput")

    fvh_d = nc.dram_tensor("fvh", [BL, s_dev * Tg], F32, kind="ExternalOutput")
    fvfin_d = nc.dram_tensor("fvfin", [BL, Tg], F32, kind="ExternalOutput")
    # bp partitions: p = b*16 + tc  (tchunks of s_dev/16)
    tcs = s_dev // 16                      # t per chunk
    bp_d = nc.dram_tensor("bp", [128, tcs * Tg], F32, kind="ExternalOutput")

    # ---- scratch ---------------------------------------------------------
    gx_d = nc.dram_tensor("gx_scr", [2, bt, G4], F32R)
    hT_d = nc.dram_tensor("hT_scr", [2, 4, 128, bt], F32R)
    feats_d = nc.dram_tensor("feats_scr", [bt, Tg], F32)

    with tile.TileContext(nc) as tc:
        with ExitStack() as octx:
            # persistent across phases
            perm = octx.enter_context(tc.tile_pool(name="perm", bufs=1))
            WhhT = perm.tile([128, 4 * G4 * 2], F32R)   # [k*2048 + g] per dir halves
            for d in range(2):
                for k in range(4):
                    nc.sync.dma_start(
                        WhhT[:, d * 4 * G4 + k * G4:(d * 4 * G4) + (k + 1) * G4],
                        WhhT_d.ap()[d * H2 + k * 128: d * H2 + (k + 1) * 128, :])
            ones1 = perm.tile([1, 128], F32R)
            nc.sync.dma_start(ones1[:], ones_d.ap())
            id8 = perm.tile([BL, BL], F32R)
            nc.sync.dma_start(id8[:], id8_d.ap())

            # ================= P1: x-projection ===========================
            with ExitStack() as ctx:
                xpool = ctx.enter_context(tc.tile_pool(name="p1x", bufs=1))
                wpool = ctx.enter_context(tc.tile_pool(name="p1w", bufs=1))
                spool = ctx.enter_context(tc.tile_pool(name="p1s", bufs=3))
                ppool = ctx.enter_context(tc.tile_pool(name="p1p", bufs=2, space="PSUM"))

                WihT = wpool.tile([128, 4 * G4 * 2], F32R)
                for d in range(2):
                    for k in range(4):
                        nc.sync.dma_start(
                            WihT[:, d * 4 * G4 + k * G4: d * 4 * G4 + (k + 1) * G4],
                            WihT_d.ap()[d * E + k * 128: d * E + (k + 1) * 128, :])
                biasg = wpool.tile([1, 2 * G4], F32R)
                nc.sync.dma_start(biasg[:], bias_d.ap().rearrange("a b -> (a b)").unsqueeze(0))

                for btc in range(bt // 128):
                    xts = []
                    for k in range(4):
                        xt = xpool.tile([128, 128], F32R, tag="xts", bufs=8)
                        nc.sync.dma_start(xt[:], xT_d.ap()[k * 128:(k + 1) * 128,
                                                           btc * 128:(btc + 1) * 128])
                        xts.append(xt)
                    for d in range(2):
                        for n in range(4):
                            ps = ppool.tile([128, 512], F32, tag="xp")
                            for k in range(4):
                                nc.tensor.matmul(
                                    ps[:],
                                    xts[k][:],
                                    WihT[:, d * 4 * G4 + k * G4 + n * 512: d * 4 * G4 + k * G4 + (n + 1) * 512],
                                    start=(k == 0), stop=False)
                            nc.tensor.matmul(
                                ps[:], ones1[:, 0:128],
                                biasg[:, d * G4 + n * 512: d * G4 + (n + 1) * 512],
                                start=False, stop=True)
                            gxs = spool.tile([128, 512], F32R, tag="gxs")
                            nc.scalar.copy(gxs[:], ps[:])
                            nc.sync.dma_start(
                                gx_d.ap()[d, btc * 128:(btc + 1) * 128, n * 512:(n + 1) * 512],
                                gxs[:])

            # ================= P2: recurrence =============================
            with ExitStack() as ctx:
                st = ctx.enter_context(tc.tile_pool(name="p2st", bufs=1))
                gxp = ctx.enter_context(tc.tile_pool(name="p2gx", bufs=2))
                gp = ctx.enter_context(tc.tile_pool(name="p2g", bufs=2))
                tp = ctx.enter_context(tc.tile_pool(name="p2t", bufs=2))
                hp = ctx.enter_context(tc.tile_pool(name="p2h", bufs=2))
                pp = ctx.enter_context(tc.tile_pool(name="p2p", bufs=1, space="PSUM"))

                hT = [st.tile([128, 4 * BL], F32R, name=f"hT{d}") for d in range(2)]
                c_t = [st.tile([BL, H2], F32, name=f"ct{d}") for d in range(2)]
                for d in range(2):
                    nc.vector.memset(hT[d][:].bitcast(F32), 0.0)
                    nc.vector.memset(c_t[d][:], 0.0)

                def step(d, t_expr):
                    """one LSTM step for direction d at time t (ScalarValue or int)."""
                    gxt = gxp.tile([BL, G4], F32R, tag=f"gx{d}")
                    nc.sync.dma_start(gxt[:], gx_d.ap()[d, ds(t_expr * BL, BL), :])
                    ps = pp.tile([BL, G4], F32, tag=f"ps{d}")
                    for n in range(4):
                        for k in range(4):
                            nc.tensor.matmul(
                                ps[:, n * 512:(n + 1) * 512],
                                hT[d][:, k * BL:(k + 1) * BL],
                                WhhT[:, d * 4 * G4 + k * G4 + n * 512: d * 4 * G4 + k * G4 + (n + 1) * 512],
                                start=(k == 0), stop=(k == 3))
                    g = gp.tile([BL, G4], F32, tag=f"g{d}")
                    nc.vector.tensor_add(g[:], ps[:], gxt[:])
                    nc.scalar.activation(g[:, 0:3 * H2], g[:, 0:3 * H2], AF.Sigmoid)
                    nc.scalar.activation(g[:, SL_G], g[:, SL_G], AF.Tanh)
                    tmp = tp.tile([BL, H2], F32, tag=f"tmp{d}")
                    nc.vector.tensor_mul(tmp[:], g[:, SL_I], g[:, SL_G])
                    nc.vector.tensor_mul(c_t[d][:], g[:, SL_F], c_t[d][:])
                    nc.vector.tensor_add(c_t[d][:], c_t[d][:], tmp[:])
                    tanhc = tp.tile([BL, H2], F32, tag=f"th{d}")
                    nc.scalar.activation(tanhc[:], c_t[d][:], AF.Tanh)
                    h = hp.tile([BL, H2], F32R, tag=f"h{d}")
                    nc.vector.tensor_mul(h[:], g[:, SL_O], tanhc[:])
                    psT = pp.tile([128, 4 * BL], F32R, tag=f"ps{d}")
                    for k in range(4):
                        nc.tensor.transpose(psT[:, k * BL:(k + 1) * BL],
                                            h[:, k * 128:(k + 1) * 128], id8[:])
                    nc.scalar.copy(hT[d][:], psT[:])
                    for k in range(4):
                        nc.sync.dma_start(
                            hT_d.ap()[d, k, :, ds(t_expr * BL, BL)],
                            hT[d][:, k * BL:(k + 1) * BL])

                niter = s_dev // u_unroll
                hints = (mybir.EngineType.PE,) if u_unroll >= 8 else ()
                with tc.For_i(0, niter, 1, staggered_reset=staggered,
                              hint_engines=hints) as i:
                    for u in range(u_unroll):
                        step(0, i * u_unroll + u)
                        step(1, (s_dev - 1) - (i * u_unroll + u))

            # ================= P3: output projection ======================
            with ExitStack() as ctx:
                wpool = ctx.enter_context(tc.tile_pool(name="p3w", bufs=1))
                lpool = ctx.enter_context(tc.tile_pool(name="p3l", bufs=3))
                spool = ctx.enter_context(tc.tile_pool(name="p3s", bufs=3))
                ppool = ctx.enter_context(tc.tile_pool(name="p3p", bufs=2, space="PSUM"))

                WoT = wpool.tile([128, 8 * Tg], F32R)      # (d*4+k) at cols *Tg
                for d in range(2):
                    for k in range(4):
                        nc.sync.dma_start(
                            WoT[:, (d * 4 + k) * Tg:(d * 4 + k + 1) * Tg],
                            WoT_d.ap()[d * H2 + k * 128: d * H2 + (k + 1) * 128, :])

                for btc in range(bt // 128):
                    ps = ppool.tile([128, Tg], F32, tag="fp")
                    for d in range(2):
                        for k in range(4):
                            hTl = lpool.tile([128, 128], F32R, tag="hTl")
                            nc.sync.dma_start(
                                hTl[:], hT_d.ap()[d, k, :, btc * 128:(btc + 1) * 128])
                            nc.tensor.matmul(ps[:], hTl[:],
                                             WoT[:, (d * 4 + k) * Tg:(d * 4 + k + 1) * Tg],
                                             start=(d == 0 and k == 0), stop=(d == 1 and k == 3))
                    fe = spool.tile([128, Tg], F32, tag="fe")
                    nc.scalar.copy(fe[:], ps[:])
                    nc.sync.dma_start(feats_d.ap()[btc * 128:(btc + 1) * 128, :], fe[:])

            # ================= P4: Viterbi DP =============================
            with ExitStack() as ctx:
                vp = ctx.enter_context(tc.tile_pool(name="p4", bufs=1))
                vt = ctx.enter_context(tc.tile_pool(name="p4t", bufs=1))

                feats_sb = vp.tile([BL, s_dev * Tg], F32)
                nc.sync.dma_start(
                    feats_sb[:].rearrange("b (t i) -> b t i", i=Tg),
                    feats_d.ap().rearrange("(t b) i -> b t i", b=BL))
                trans_sb = vp.tile([128, Tg * Tg], F32)
                nc.sync.dma_start(trans_sb[:], trans_d.ap())
                revio_sb = vp.tile([128, Tg * Tg], F32)
                nc.sync.dma_start(revio_sb[:], revio_d.ap())
                fv = vp.tile([BL, Tg], F32)
                nc.sync.dma_start(fv[:], fv0_d.ap())
                fvh_sb = vp.tile([BL, s_dev * Tg], F32)

                uv = 8
                with tc.For_i(0, s_dev // uv, 1) as i:
                    for u in range(uv):
                        t = i * uv + u
                        nc.scalar.copy(fvh_sb[:, ds(t * Tg, Tg)], fv[:])
                        nv = vt.tile([BL, Tg * Tg], F32, tag="nv")
                        nc.vector.tensor_tensor(
                            nv[:].rearrange("b (i j) -> b i j", i=Tg),
                            fv[:].unsqueeze(1).broadcast_to([BL, Tg, Tg]),
                            trans_sb[0:BL, :].rearrange("b (i j) -> b i j", i=Tg),
                            OP.add)
                        nc.vector.tensor_reduce(
                            fv[:], nv[:].rearrange("b (i j) -> b i j", i=Tg),
                            mybir.AxisListType.X, OP.max)
                        nc.vector.tensor_add(fv[:], fv[:], feats_sb[:, ds(t * Tg, Tg)])
                nc.sync.dma_start(fvh_d.ap(), fvh_sb[:])
                nc.sync.dma_start(fvfin_d.ap(), fv[:])

                # ============= P5: backpointers (resharded) ===============
                fvp = vp.tile([128, tcs * Tg], F32)     # p = b*16+tc
                for b in range(BL):
                    nc.sync.dma_start(
                        fvp[b * 16:(b + 1) * 16, :],
                        fvh_d.ap().rearrange("b (tc r) -> b tc r", tc=16)[b, :, :])
                nva = vt.tile([128, tcs * Tg * Tg], F32, tag="nva")
                nc.vector.tensor_tensor(
                    nva[:].rearrange("p (t i j) -> p t i j", t=tcs, i=Tg),
                    fvp[:].rearrange("p (t j) -> p t j", t=tcs).unsqueeze(2).broadcast_to([128, tcs, Tg, Tg]),
                    trans_sb[:].rearrange("p (i j) -> p i j", i=Tg).unsqueeze(1).broadcast_to([128, tcs, Tg, Tg]),
                    OP.add)
                mx = vt.tile([128, tcs * Tg], F32, tag="mx")
                nc.vector.tensor_reduce(
                    mx[:], nva[:].rearrange("p (t i j) -> p t i j", t=tcs, i=Tg),
                    mybir.AxisListType.X, OP.max)
                eq = vt.tile([128, tcs * Tg * Tg], F32, tag="eq")
                nc.vector.tensor_tensor(
                    eq[:].rearrange("p (t i j) -> p t i j", t=tcs, i=Tg),
                    nva[:].rearrange("p (t i j) -> p t i j", t=tcs, i=Tg),
                    mx[:].rearrange("p (t i) -> p t i", t=tcs).unsqueeze(3).broadcast_to([128, tcs, Tg, Tg]),
                    OP.is_equal)
                nc.vector.tensor_tensor(
                    eq[:].rearrange("p (t i j) -> p t i j", t=tcs, i=Tg),
                    eq[:].rearrange("p (t i j) -> p t i j", t=tcs, i=Tg),
                    revio_sb[:].rearrange("p (i j) -> p i j", i=Tg).unsqueeze(1).broadcast_to([128, tcs, Tg, Tg]),
                    OP.mult)
                bpv = vt.tile([128, tcs * Tg], F32, tag="bpv")
                nc.vector.tensor_reduce(
                    bpv[:], eq[:].rearrange("p (t i j) -> p t i j", t=tcs, i=Tg),
                    mybir.AxisListType.X, OP.max)
                nc.sync.dma_start(bp_d.ap(), bpv[:])

    nc.compile()
    return nc


# ---------------------------------------------------------------------------
_NC_CACHE = {}


def _get_nc(s_dev=S):
    if s_dev not in _NC_CACHE:
        _NC_CACHE[s_dev] = build_nc(s_dev)
    return _NC_CACHE[s_dev]


def host_prep(inputs, s_dev=S):
    """Build per-core in_maps from full inputs."""
    sent = np.asarray(inputs["sentences"])
    emb = np.asarray(inputs["emb"], dtype=np.float32)
    f32 = lambda k: np.asarray(inputs[k], dtype=np.float32)
    W_ih = [f32("W_ih_f"), f32("W_ih_b")]
    W_hh = [f32("W_hh_f"), f32("W_hh_b")]
    b_g = [f32("b_f"), f32("b_b")]
    W_out = f32("W_out")
    b_out = f32("b_out")
    trans = f32("trans")

    # gate perm [i, f, g, o] -> [i, f, o, g]
    def gperm(M):
        return np.concatenate([M[:H2], M[H2:2 * H2], M[3 * H2:], M[2 * H2:3 * H2]], axis=0)

    WihT = np.concatenate([gperm(W_ih[d]).T for d in range(2)], axis=0)       # [2E, G4]
    WhhT = np.concatenate([gperm(W_hh[d]).T for d in range(2)], axis=0)       # [2H2, G4]
    biasg = np.stack([gperm(b_g[d].reshape(G4, 1))[:, 0] for d in range(2)])  # [2, G4]
    WoT = np.concatenate([W_out[:, :H2].T, W_out[:, H2:].T], axis=0)          # [2H2, Tg]
    transf = np.tile((trans + b_out[:, None]).reshape(1, -1), (128, 1)).astype(np.float32)
    revio = np.tile(np.tile(Tg - 1.0 - np.arange(Tg, dtype=np.float32), Tg).reshape(1, -1), (128, 1))
    fv0 = np.full((BL, Tg), NEG, np.float32)
    fv0[:, START] = 0.0
    ones1 = np.ones((1, 128), np.float32)
    id8 = np.eye(BL, dtype=np.float32)

    x = emb[sent]                                   # [B, S, E]
    in_maps = []
    for c in range(NCORES):
        xc = x[c * BL:(c + 1) * BL, :s_dev]         # [BL, s, E]
        xT = np.ascontiguousarray(xc.transpose(2, 1, 0).reshape(E, s_dev * BL))
        in_maps.append(dict(
            xT=xT.astype(np.float32), WihT=WihT, WhhT=WhhT, biasg=biasg,
            ones1=ones1, id8=id8, WoT=WoT, transf=transf, revio=revio, fv0=fv0))
    return in_maps, trans


def host_post(results, trans, s_dev=S):
    """Assemble (scores, paths) from per-core outputs."""
    tcs = s_dev // 16
    scores = np.empty((B,), np.float32)
    paths = np.empty((B, s_dev), np.int32)
    for c, r in enumerate(results):
        fvfin = r["fvfin"]                          # [BL, Tg]
        bpv = r["bp"]                               # [128, tcs*Tg]
        bp = np.empty((BL, s_dev, Tg), np.int64)
        for b in range(BL):
            for tchunk in range(16):
                blk = bpv[b * 16 + tchunk].reshape(tcs, Tg)
                bp[b, tchunk * tcs:(tchunk + 1) * tcs] = (Tg - 1.0 - blk).astype(np.int64)
        term = fvfin + trans[STOP][None, :]         # [BL, Tg]
        for b in range(BL):
            gb = c * BL + b
            best = int(np.argmax(term[b]))
            scores[gb] = term[b, best]
            tag = best
            for t in range(s_dev - 1, -1, -1):
                paths[gb, t] = tag
                tag = int(bp[b, t, tag])
    return scores, paths


def kernel(**inputs):
    nc = _get_nc(S)
    in_maps, trans = host_prep(inputs, S)
    res = bass_utils.run_bass_kernel_spmd(nc, in_maps, core_ids=list(range(NCORES)))
    return host_post(res.results, trans, S)
